# revision 13
# baseline (speedup 1.0000x reference)
"""Fully-fused AxialAttention kernel for TRN2 (8 cores, data-parallel over N).

Per core: 2 images. Layout: free f = n*4096 + h*64 + w  (h = attended axis).
Per head h (8 heads, 128 oc each: q' 0-31 / k' 32-63 / v 64-127):
  qkv = BN(W x) with per-partition affine at psum evict (a1 folded into q').
  scores ST[j,i] per bg=(n,w), chunked as (w-parity, j) x 128 partitions:
    kr via per-j batched matmuls + per-chunk PE-transpose-accumulate,
    qk via per-bg matmuls (operand swap -> transposed orientation),
    qr via per-i batched matmuls, added at evict (scalar_tensor_tensor).
  softmax over j (= partitions) via ones-matmul + reciprocal + DMA-broadcast.
  sv via per-bg matmuls on PE-transposed v; sve via per-i batched matmuls
  against host-precomputed v_emb^T; output affine (bno folded) at evict.
"""
import sys

for _p in ("/opt/trn_rl_repo", "/root/.axon_site/_ro/trn_rl_repo"):
    if _p not in sys.path:
        sys.path.append(_p)

from contextlib import ExitStack

import numpy as np
import ml_dtypes

import concourse.bass as bass
import concourse.mybir as mybir
import concourse.tile as tile
from concourse import bacc
from concourse.bass_utils import run_bass_kernel_spmd

BF16 = ml_dtypes.bfloat16
FP32 = mybir.dt.float32
BF = mybir.dt.bfloat16

N, C, H, W = 16, 512, 64, 64
NH = 8            # heads
GP = 64           # planes per head (v); q/k have 32 each
BN_EPS = 1e-5
NCORES = 8
NPC = N // NCORES  # 2 images per core
F = NPC * H * W    # 8192 free columns
AF = mybir.ActivationFunctionType
ALU = mybir.AluOpType


def _emit(tc, xi, wt, scl, bia, rel_t, v_embT, ones2, idb, idf, aE, aO, bb, sc2, oe):
    """Emit the full per-core program into TileContext tc.

    xi: dram x (2, 512, 64, 64) bf16      wt: dram (512, 1024) bf16
    scl/bia: dram (128, 8) f32            rel_t: dram (64, 127) bf16
    v_embT: dram (128, 4096) bf16         ones2: dram (128, 2) bf16
    idb: dram (128, 64) bf16              idf: dram (64, 64) f32
    aE/aO/bb: dram (128, 8) f32           sc2: dram (1, 8) f32 [a2/a1, a3 packed (2,8)->(1,16)]
    oe: dram out (2, 512, 64, 64) bf16
    """
    nc = tc.nc
    ctx = ExitStack()
    with ctx:
        konst = ctx.enter_context(tc.tile_pool(name="konst", bufs=1))
        big = ctx.enter_context(tc.tile_pool(name="big", bufs=1))
        med = ctx.enter_context(tc.tile_pool(name="med", bufs=2))
        ps2 = ctx.enter_context(tc.tile_pool(name="ps2", bufs=1, space="PSUM"))

        # ---- constants into SBUF ----
        wts = []
        for k in range(4):
            w_t = konst.tile([128, 1024], BF, tag=f"wt{k}")
            nc.sync.dma_start(out=w_t[:], in_=wt[128 * k:128 * (k + 1), :])
            wts.append(w_t)
        scl_s = konst.tile([128, 8], FP32, tag="scl")
        nc.sync.dma_start(out=scl_s[:], in_=scl[:])
        bia_s = konst.tile([128, 8], FP32, tag="bia")
        nc.sync.dma_start(out=bia_s[:], in_=bia[:])
        aE_s = konst.tile([128, 8], FP32, tag="aE")
        nc.sync.dma_start(out=aE_s[:], in_=aE[:])
        aO_s = konst.tile([128, 8], FP32, tag="aO")
        nc.sync.dma_start(out=aO_s[:], in_=aO[:])
        bb_s = konst.tile([128, 8], FP32, tag="bb")
        nc.sync.dma_start(out=bb_s[:], in_=bb[:])
        rel_s = konst.tile([64, 127], BF, tag="rel")
        nc.sync.dma_start(out=rel_s[:], in_=rel_t[:])
        vet_s = konst.tile([128, 4096], BF, tag="vet")
        nc.sync.dma_start(out=vet_s[:], in_=v_embT[:])
        on2_s = konst.tile([128, 33], BF, tag="on2")
        nc.sync.dma_start(out=on2_s[:], in_=ones2[:])
        idb_s = konst.tile([128, 64], BF, tag="idb")
        nc.sync.dma_start(out=idb_s[:], in_=idb[:])
        idf_s = konst.tile([64, 64], FP32, tag="idf")
        nc.sync.dma_start(out=idf_s[:], in_=idf[:])
        onesb = konst.tile([1, 64], BF, tag="onesb")
        nc.vector.memset(onesb[:], 1.0)

        # host scalars shipped through numpy (sc2 unused on device; scales
        # a2/a1, a3 are baked into evict scalars at build time via python
        # floats passed in sc2_np (see _build); here sc2 is a placeholder).

        xr = xi.rearrange("n c (hh h2) w -> c n hh (h2 w)", hh=2)  # (512,2,2,2048)
        oer = oe.rearrange("n c h w -> c n (h w)")          # (512, 2, 4096)

        for h in range(NH):
            qkv = big.tile([128, F], BF, tag="qkv")
            qkvr = qkv.rearrange("p (n h w) -> p n h w", n=2, h=64)
            qkvw = qkv.rearrange("p (n h w) -> p n w h", n=2, h=64)

            # ---------- projection ----------
            for q in range(4):          # quarters of F
                nn, hh = q // 2, q % 2
                xq = []
                for k in range(4):
                    xt = med.tile([128, 2048], BF, tag=f"xq{k}")
                    nc.sync.dma_start(out=xt[:], in_=xr[128 * k:128 * (k + 1), nn, hh, :])
                    xq.append(xt)
                for s in range(4):      # 512-col sub-chunks
                    pp = ps2.tile([128, 512], FP32, tag="proj")
                    for k in range(4):
                        nc.tensor.matmul(
                            pp[:], lhsT=wts[k][:, 128 * h:128 * (h + 1)],
                            rhs=xq[k][:, 512 * s:512 * (s + 1)],
                            start=(k == 0), stop=(k == 3))
                    nc.scalar.activation(
                        qkv[:, 2048 * q + 512 * s: 2048 * q + 512 * (s + 1)], pp[:],
                        AF.Identity, bias=bia_s[:, h:h + 1], scale=scl_s[:, h:h + 1])

            # ---------- ext = [k' at 0:32 | q' at 32:64] ----------
            ext = big.tile([64, F], BF, tag="ext")
            nc.vector.tensor_copy(ext[0:32, :], qkv[32:64, :])
            nc.vector.tensor_copy(ext[32:64, :], qkv[0:32, :])
            extr = ext.rearrange("p (n h w) -> p n h w", n=2, h=64)

            # ---------- vT (PE pair-transposes) ----------
            vT = big.tile([128, 4096], BF, tag="vT")
            for b8 in range(8):         # 8 banks x 8 pairs
                pv = ps2.tile([128, 512], FP32, tag="proj")
                for tt in range(8):
                    t = b8 * 8 + tt
                    nn, wp = t // 32, t % 32
                    for par in range(2):
                        nc.tensor.matmul(
                            pv[64 * par:64 * (par + 1), 64 * tt:64 * (tt + 1)],
                            lhsT=qkvr[64:128, nn, :, 2 * wp + par],
                            rhs=idb_s[64:128, :],
                            start=True, stop=True)
                nc.vector.tensor_copy(vT[:, 512 * b8:512 * (b8 + 1)], pv[:])

            # ---------- qr (per-i batched) ----------
            qr_raw = big.tile([64, F], FP32, tag="qr")
            a21 = float(_emit.a2[h] / _emit.a1[h])
            for i4 in range(16):
                pq = ps2.tile([64, 512], FP32, tag="qrkr")
                for ii in range(4):
                    i = i4 * 4 + ii
                    nc.tensor.matmul(
                        pq[:, 128 * ii:128 * (ii + 1)],
                        lhsT=rel_s[0:32, 63 - i:127 - i],
                        rhs=qkvr[0:32, :, i, :], start=True, stop=True)
                nc.vector.tensor_scalar_mul(
                    qr_raw[:, 512 * i4:512 * (i4 + 1)], pq[:], a21)

            # ---------- kr (per-j batched) ----------
            kr_sb = big.tile([64, F], FP32, tag="kr")
            a3 = float(_emit.a3[h])
            for j4 in range(16):
                pk = ps2.tile([64, 512], FP32, tag="qrkr")
                for jj in range(4):
                    j = j4 * 4 + jj
                    nc.tensor.matmul(
                        pk[:, 128 * jj:128 * (jj + 1)],
                        lhsT=rel_s[32:64, 63 - j:127 - j],
                        rhs=qkvr[32:64, :, j, :], start=True, stop=True)
                nc.vector.tensor_scalar_mul(
                    kr_sb[:, 512 * j4:512 * (j4 + 1)], pk[:], a3)
            krr = kr_sb.rearrange("p (j n w) -> p j n w", j=64, n=2)
            qrw = qr_raw.rearrange("p (i n w) -> p n w i", i=64, n=2)

            # ---------- scores + softmax ----------
            E = big.tile([128, 4096], BF, tag="E")
            En = big.tile([128, 4096], BF, tag="En")
            for B in range(8):          # banks of 8 chunks; n = B // 4
                nn = B // 4
                wp0 = (B % 4) * 8
                pST = ps2.tile([128, 512], FP32, tag="ST")
                for tt in range(8):
                    wp = wp0 + tt
                    blk = pST[:, 64 * tt:64 * (tt + 1)]
                    # kr^T accumulate (f32 matmul against identity), per bg
                    nc.tensor.matmul(
                        blk[0:64, :], lhsT=krr[0:64, :, nn, 2 * wp],
                        rhs=idf_s[:], start=True, stop=False,
                        skip_group_check=True)
                    nc.tensor.matmul(
                        blk[64:128, :], lhsT=krr[0:64, :, nn, 2 * wp + 1],
                        rhs=idf_s[:], start=True, stop=False,
                        skip_group_check=True)
                    # qk even bg (w = 2wp): k' home, q' from ext (base 32)
                    nc.tensor.matmul(
                        blk[0:64, :], lhsT=qkvr[32:64, nn, :, 2 * wp],
                        rhs=extr[32:64, nn, :, 2 * wp],
                        start=False, stop=True, skip_group_check=True)
                    # qk odd bg (w = 2wp+1): k' copy (base 0), q' home
                    nc.tensor.matmul(
                        blk[64:128, :], lhsT=extr[0:32, nn, :, 2 * wp + 1],
                        rhs=qkvr[0:32, nn, :, 2 * wp + 1],
                        start=False, stop=True, skip_group_check=True)
                S_u = med.tile([128, 512], FP32, tag="Su")
                nc.vector.scalar_tensor_tensor(
                    S_u[0:64, :], pST[0:64, :], 1.0,
                    qrw[0:64, nn, 2 * wp0:2 * wp0 + 16:2, :],
                    op0=ALU.mult, op1=ALU.add)
                nc.vector.scalar_tensor_tensor(
                    S_u[64:128, :], pST[64:128, :], 1.0,
                    qrw[0:64, nn, 2 * wp0 + 1:2 * wp0 + 16:2, :],
                    op0=ALU.mult, op1=ALU.add)
                nc.scalar.activation(E[:, 512 * B:512 * (B + 1)], S_u[:], AF.Exp)
                # Z = column sums over j per half, 1/Z, broadcast via K=1 matmul
                pZ = ps2.tile([33, 512], FP32, tag="Z")
                nc.tensor.matmul(pZ[:], lhsT=on2_s[:], rhs=E[:, 512 * B:512 * (B + 1)],
                                 start=True, stop=True)
                Zlo = med.tile([1, 512], BF, tag="Zlo")
                Zhi = med.tile([1, 512], BF, tag="Zhi")
                with nc.allow_low_precision(reason="softmax 1/Z in bf16 is fine"):
                    nc.vector.reciprocal(Zlo[:], pZ[0:1, :])
                    nc.vector.reciprocal(Zhi[:], pZ[32:33, :])
                pZb = ps2.tile([128, 512], FP32, tag="Zb")
                nc.tensor.matmul(pZb[0:64, :], lhsT=onesb[:], rhs=Zlo[:],
                                 start=True, stop=True)
                nc.tensor.matmul(pZb[64:128, :], lhsT=onesb[:], rhs=Zhi[:],
                                 start=True, stop=True)
                nc.vector.tensor_mul(En[:, 512 * B:512 * (B + 1)],
                                     E[:, 512 * B:512 * (B + 1)], pZb[:])

            Eni = En.rearrange("p (t i) -> p i t", t=64)
            O = big.tile([64, F], BF, tag="O")
            Osv = O.rearrange("p (n h w) -> p n w h", n=2, h=64)
            Osve = O.rearrange("p (n h w) -> p h n w", n=2, h=64)

            # ---------- sv ----------
            for B in range(8):          # bank: 16 bg = one n, w in [16B'..]
                nn = B // 4
                wp0 = (B % 4) * 8
                pSV = ps2.tile([128, 512], FP32, tag="SV")
                for tt in range(8):
                    for par in range(2):
                        t = nn * 32 + wp0 + tt
                        nc.tensor.matmul(
                            pSV[64 * par:64 * (par + 1), 64 * tt:64 * (tt + 1)],
                            lhsT=vT[64 * par:64 * (par + 1), 64 * t:64 * (t + 1)],
                            rhs=En[64 * par:64 * (par + 1), 64 * t:64 * (t + 1)],
                            start=True, stop=True)
                for par in range(2):
                    nc.vector.tensor_scalar(
                        Osv[0:64, nn, 2 * wp0 + par:2 * wp0 + 16:2, :],
                        pSV[64 * par:64 * (par + 1), :].rearrange(
                            "p (t i) -> p t i", t=8),
                        aE_s[64 * par:64 * par + 64, h:h + 1],
                        bb_s[64 * par:64 * par + 64, h:h + 1],
                        op0=ALU.mult, op1=ALU.add)

            # ---------- sve ----------
            for ig in range(8):         # 8 i's x 2 par per bank
                pSE = ps2.tile([128, 512], FP32, tag="SV")
                for ii in range(8):
                    i = ig * 8 + ii
                    for par in range(2):
                        nc.tensor.matmul(
                            pSE[64 * par:64 * (par + 1), 64 * ii:64 * (ii + 1)],
                            lhsT=vet_s[64 * par:64 * (par + 1), 64 * i:64 * (i + 1)],
                            rhs=Eni[64 * par:64 * (par + 1), i, :],
                            start=True, stop=True)
                for par in range(2):
                    for nn2 in range(2):
                        nc.vector.scalar_tensor_tensor(
                            Osve[0:64, 8 * ig:8 * (ig + 1), nn2, par::2],
                            pSE[64 * par:64 * (par + 1), :].rearrange(
                                "p (ii n wp) -> p ii n wp", ii=8, n=2)[:, :, nn2, :],
                            aO_s[0:64, h:h + 1],
                            Osve[0:64, 8 * ig:8 * (ig + 1), nn2, par::2],
                            op0=ALU.mult, op1=ALU.add)

            # ---------- DMA out ----------
            Ond = O.rearrange("p (n hw) -> p n hw", n=2)
            nc.sync.dma_start(out=oer[64 * h:64 * (h + 1), :, :], in_=Ond[:, :, :])


def _fold_constants(w_qkv, relative,
                    bnq_g, bnq_b, bnq_m, bnq_v,
                    bns_g, bns_b, bns_m, bns_v,
                    bno_g, bno_b, bno_m, bno_v):
    s_q = bnq_g / np.sqrt(bnq_v + BN_EPS)
    t_q = bnq_b - bnq_m * s_q
    a_s = bns_g / np.sqrt(bns_v + BN_EPS)
    a1, a2, a3 = a_s[0:8], a_s[8:16], a_s[16:24]
    s_o = bno_g / np.sqrt(bno_v + BN_EPS)
    t_o = bno_b - bno_m * s_o

    scl = np.empty((128, 8), np.float32)
    bia = np.empty((128, 8), np.float32)
    for h in range(8):
        oc = h * 128 + np.arange(128)
        f = np.where(np.arange(128) < 32, a1[h], 1.0)
        scl[:, h] = s_q[oc] * f
        bia[:, h] = t_q[oc] * f

    rel_t = np.empty((64, 127), np.float32)
    rel_t[0:32] = relative[0:32, ::-1]       # relq reversed
    rel_t[32:64] = relative[32:64, ::-1]     # relk reversed

    # v_embT[par*64+j, i*64+c] = relative[64+c, i-j+63]
    iidx = np.arange(64)
    v_embT = np.empty((128, 4096), np.float32)
    jj, ii_, cc = np.meshgrid(np.arange(64), iidx, np.arange(64), indexing="ij")
    tbl = relative[64 + cc, ii_ - jj + 63]   # (j, i, c)
    v_embT[0:64] = tbl.reshape(64, 4096)
    v_embT[64:128] = tbl.reshape(64, 4096)

    ones2 = np.zeros((128, 33), np.float32)
    ones2[0:64, 0] = 1.0
    ones2[64:128, 32] = 1.0
    idb = np.zeros((128, 64), np.float32)
    idb[np.arange(128), np.arange(128) % 64] = 1.0
    idf = np.eye(64, dtype=np.float32)

    aE = np.empty((128, 8), np.float32)
    aO = np.empty((128, 8), np.float32)
    bb = np.empty((128, 8), np.float32)
    for h in range(8):
        cc_ = np.arange(128) % 64
        oc2 = (h * 64 + cc_) * 2
        aE[:, h] = s_o[oc2]
        aO[:, h] = s_o[oc2 + 1]
        bb[:, h] = t_o[oc2] + t_o[oc2 + 1]

    wt = np.ascontiguousarray(w_qkv.T)
    return dict(wt=wt.astype(BF16), scl=scl, bia=bia,
                rel_t=np.ascontiguousarray(rel_t).astype(BF16),
                v_embT=v_embT.astype(BF16), ones2=ones2.astype(BF16),
                idb=idb.astype(BF16), idf=idf,
                aE=aE, aO=aO, bb=bb, a1=a1, a2=a2, a3=a3)


def _build_graph(a1, a2, a3):
    nc = bacc.Bacc("TRN2")
    xi = nc.dram_tensor("x", (2, 512, 64, 64), BF, kind="ExternalInput")
    wt = nc.dram_tensor("wt", (512, 1024), BF, kind="ExternalInput")
    scl = nc.dram_tensor("scl", (128, 8), FP32, kind="ExternalInput")
    bia = nc.dram_tensor("bia", (128, 8), FP32, kind="ExternalInput")
    rel_t = nc.dram_tensor("rel_t", (64, 127), BF, kind="ExternalInput")
    v_embT = nc.dram_tensor("v_embT", (128, 4096), BF, kind="ExternalInput")
    ones2 = nc.dram_tensor("ones2", (128, 33), BF, kind="ExternalInput")
    idb = nc.dram_tensor("idb", (128, 64), BF, kind="ExternalInput")
    idf = nc.dram_tensor("idf", (64, 64), FP32, kind="ExternalInput")
    aE = nc.dram_tensor("aE", (128, 8), FP32, kind="ExternalInput")
    aO = nc.dram_tensor("aO", (128, 8), FP32, kind="ExternalInput")
    bb = nc.dram_tensor("bb", (128, 8), FP32, kind="ExternalInput")
    sc2 = nc.dram_tensor("sc2", (1, 8), FP32, kind="ExternalInput")
    oe = nc.dram_tensor("o", (2, 512, 64, 64), BF, kind="ExternalOutput")

    _emit.a1, _emit.a2, _emit.a3 = a1, a2, a3
    with tile.TileContext(nc) as tc:
        _emit(tc, xi[:], wt[:], scl[:], bia[:], rel_t[:], v_embT[:], ones2[:],
              idb[:], idf[:], aE[:], aO[:], bb[:], sc2[:], oe[:])
    nc.compile()
    return nc


_LAST_EXEC_NS = None


def _mesh_sharding():
    import jax
    from jax.sharding import Mesh, PartitionSpec, NamedSharding
    devices = jax.devices()[:NCORES]
    mesh = Mesh(np.asarray(devices), ("core",))
    return mesh, NamedSharding(mesh, PartitionSpec("core"))


def _run_spmd_fast(nc, global_ins):
    """Execute nc on 8 cores via shard_map with on-device zero outputs and
    pre-transferred (async) inputs. global_ins: name -> global array whose
    axis 0 is ncores * per_core_dim0."""
    import jax
    import jax.numpy as jnp
    from jax.sharding import Mesh, PartitionSpec, NamedSharding
    from concourse import bass2jax

    bass2jax.install_neuronx_cc_hook()
    partition_name = (nc.partition_id_tensor.name
                      if nc.partition_id_tensor else None)
    in_names, out_names, out_avals = [], [], []
    for alloc in nc.m.functions[0].allocations:
        if not isinstance(alloc, mybir.MemoryLocationSet):
            continue
        name = alloc.memorylocations[0].name
        if alloc.kind == "ExternalInput":
            if name != partition_name:
                in_names.append(name)
        elif alloc.kind == "ExternalOutput":
            shape = tuple(alloc.tensor_shape)
            dtype = mybir.dt.np(alloc.dtype)
            out_names.append(name)
            out_avals.append(jax.core.ShapedArray(shape, dtype))
    n_params = len(in_names)
    n_outs = len(out_avals)
    # No zero output buffers: this kernel writes every output element, so
    # uninitialized PJRT result buffers are fine (saves their wire transfer).
    all_in_names = list(in_names)
    if partition_name is not None:
        all_in_names.append(partition_name)

    mesh, sh = _mesh_sharding()

    # inputs were device_put by the caller (async, overlapped with build)
    dev_in = [global_ins[nm] for nm in in_names]

    def _body(*args):
        operands = list(args)
        if partition_name is not None:
            operands.append(bass2jax.partition_id_tensor())
        outs = bass2jax._bass_exec_p.bind(
            *operands,
            out_avals=tuple(out_avals),
            in_names=tuple(all_in_names),
            out_names=tuple(out_names),
            lowering_input_output_aliases=(),
            sim_require_finite=True,
            sim_require_nnan=True,
            nc=nc,
        )
        return tuple(outs)

    in_specs = (PartitionSpec("core"),) * n_params
    out_specs = (PartitionSpec("core"),) * n_outs
    sharded = jax.jit(
        bass2jax.shard_map(_body, mesh=mesh, in_specs=in_specs,
                           out_specs=out_specs, check_rep=False),
        keep_unused=True)
    out_arrs = sharded(*dev_in)
    return {nm: np.asarray(out_arrs[i]) for i, nm in enumerate(out_names)}


def kernel(x, w_qkv, relative,
           bnq_g, bnq_b, bnq_m, bnq_v,
           bns_g, bns_b, bns_m, bns_v,
           bno_g, bno_b, bno_m, bno_v):
    global _LAST_EXEC_NS
    cst = _fold_constants(np.asarray(w_qkv, np.float32), np.asarray(relative, np.float32),
                          *[np.asarray(a, np.float32) for a in
                            (bnq_g, bnq_b, bnq_m, bnq_v, bns_g, bns_b, bns_m, bns_v,
                             bno_g, bno_b, bno_m, bno_v)])
    x_bf = np.ascontiguousarray(np.asarray(x, np.float32)).astype(BF16)
    global_ins = {"x": x_bf, "sc2": np.zeros((NCORES, 8), np.float32)}
    for k in ("wt", "scl", "bia", "rel_t", "v_embT", "ones2",
              "idb", "idf", "aE", "aO", "bb"):
        a = cst[k]
        global_ins[k] = np.tile(a, (NCORES,) + (1,) * (a.ndim - 1))
    # kick off async H2D now so the transfer overlaps graph build + compile.
    # Use per-device puts + make_array (the NamedSharding device_put path is
    # drastically slower and occasionally pathological through axon).
    import jax
    _, sh = _mesh_sharding()
    devs = jax.devices()[:NCORES]
    # tiny per-device warmup: first transfer on a cold channel runs ~4x
    # slower; a small put first moves the big ones onto the warm path
    _tiny = np.ones((8, 128), BF16)
    for _w in [jax.device_put(_tiny, dv) for dv in devs]:
        _w.block_until_ready()

    def _put_global(arr):
        per = arr.shape[0] // NCORES
        shards = [jax.device_put(arr[r * per:(r + 1) * per], devs[r])
                  for r in range(NCORES)]
        return jax.make_array_from_single_device_arrays(arr.shape, sh, shards)

    global_ins = {k: _put_global(v) for k, v in global_ins.items()}
    nc = _build_graph(cst["a1"], cst["a2"], cst["a3"])
    res = _run_spmd_fast(nc, global_ins)
    _LAST_EXEC_NS = None
    return res["o"].astype(np.float32)


# revision 14
# speedup vs baseline: 9.2645x; 9.2645x over previous
"""Fully-fused AxialAttention kernel for TRN2 (8 cores, data-parallel over N).

Per core: 2 images. Layout: free f = n*4096 + h*64 + w  (h = attended axis).
Per head h (8 heads, 128 oc each: q' 0-31 / k' 32-63 / v 64-127):
  qkv = BN(W x) with per-partition affine at psum evict (a1 folded into q').
  scores ST[j,i] per bg=(n,w), chunked as (w-parity, j) x 128 partitions:
    kr via per-j batched matmuls + per-chunk PE-transpose-accumulate,
    qk via per-bg matmuls (operand swap -> transposed orientation),
    qr via per-i batched matmuls, added at evict (scalar_tensor_tensor).
  softmax over j (= partitions) via ones-matmul + reciprocal + DMA-broadcast.
  sv via per-bg matmuls on PE-transposed v; sve via per-i batched matmuls
  against host-precomputed v_emb^T; output affine (bno folded) at evict.
"""
import sys

for _p in ("/opt/trn_rl_repo", "/root/.axon_site/_ro/trn_rl_repo"):
    if _p not in sys.path:
        sys.path.append(_p)

from contextlib import ExitStack

import numpy as np
import ml_dtypes

import concourse.bass as bass
import concourse.mybir as mybir
import concourse.tile as tile
from concourse import bacc
from concourse.bass_utils import run_bass_kernel_spmd

BF16 = ml_dtypes.bfloat16
FP32 = mybir.dt.float32
BF = mybir.dt.bfloat16

N, C, H, W = 16, 512, 64, 64
NH = 8            # heads
GP = 64           # planes per head (v); q/k have 32 each
BN_EPS = 1e-5
NCORES = 8
NPC = N // NCORES  # 2 images per core
F = NPC * H * W    # 8192 free columns
AF = mybir.ActivationFunctionType
ALU = mybir.AluOpType


def _emit(tc, xi, wt, scl, bia, rel_t, v_embT, ones2, idb, idf, aE, aO, bb, sc2, oe):
    """Emit the full per-core program into TileContext tc.

    xi: dram x (2, 512, 64, 64) bf16      wt: dram (512, 1024) bf16
    scl/bia: dram (128, 8) f32            rel_t: dram (64, 127) bf16
    v_embT: dram (128, 4096) bf16         ones2: dram (128, 2) bf16
    idb: dram (128, 64) bf16              idf: dram (64, 64) f32
    aE/aO/bb: dram (128, 8) f32           sc2: dram (1, 8) f32 [a2/a1, a3 packed (2,8)->(1,16)]
    oe: dram out (2, 512, 64, 64) bf16
    """
    nc = tc.nc
    ctx = ExitStack()
    with ctx:
        konst = ctx.enter_context(tc.tile_pool(name="konst", bufs=1))
        big = ctx.enter_context(tc.tile_pool(name="big", bufs=1))
        med = ctx.enter_context(tc.tile_pool(name="med", bufs=2))
        ps2 = ctx.enter_context(tc.tile_pool(name="ps2", bufs=1, space="PSUM"))

        # ---- constants into SBUF ----
        wts = []
        for k in range(4):
            w_t = konst.tile([128, 1024], BF, tag=f"wt{k}")
            nc.sync.dma_start(out=w_t[:], in_=wt[128 * k:128 * (k + 1), :])
            wts.append(w_t)
        scl_s = konst.tile([128, 8], FP32, tag="scl")
        nc.sync.dma_start(out=scl_s[:], in_=scl[:])
        bia_s = konst.tile([128, 8], FP32, tag="bia")
        nc.sync.dma_start(out=bia_s[:], in_=bia[:])
        aE_s = konst.tile([128, 8], FP32, tag="aE")
        nc.sync.dma_start(out=aE_s[:], in_=aE[:])
        aO_s = konst.tile([128, 8], FP32, tag="aO")
        nc.sync.dma_start(out=aO_s[:], in_=aO[:])
        bb_s = konst.tile([128, 8], FP32, tag="bb")
        nc.sync.dma_start(out=bb_s[:], in_=bb[:])
        rel_s = konst.tile([64, 127], BF, tag="rel")
        nc.sync.dma_start(out=rel_s[:], in_=rel_t[:])
        vet_s = konst.tile([128, 4096], BF, tag="vet")
        nc.sync.dma_start(out=vet_s[:], in_=v_embT[:])
        on2_s = konst.tile([128, 33], BF, tag="on2")
        nc.sync.dma_start(out=on2_s[:], in_=ones2[:])
        idb_s = konst.tile([128, 64], BF, tag="idb")
        nc.sync.dma_start(out=idb_s[:], in_=idb[:])
        idf_s = konst.tile([64, 64], FP32, tag="idf")
        nc.sync.dma_start(out=idf_s[:], in_=idf[:])
        onesb = konst.tile([1, 64], BF, tag="onesb")
        nc.vector.memset(onesb[:], 1.0)

        # host scalars shipped through numpy (sc2 unused on device; scales
        # a2/a1, a3 are baked into evict scalars at build time via python
        # floats passed in sc2_np (see _build); here sc2 is a placeholder).

        xr = xi.rearrange("n c (hh h2) w -> c n hh (h2 w)", hh=2)  # (512,2,2,2048)
        oer = oe.rearrange("n c h w -> c n (h w)")          # (512, 2, 4096)

        for h in range(NH):
            qkv = big.tile([128, F], BF, tag="qkv")
            qkvr = qkv.rearrange("p (n h w) -> p n h w", n=2, h=64)
            qkvw = qkv.rearrange("p (n h w) -> p n w h", n=2, h=64)

            # ---------- projection ----------
            for q in range(4):          # quarters of F
                nn, hh = q // 2, q % 2
                xq = []
                for k in range(4):
                    xt = med.tile([128, 2048], BF, tag=f"xq{k}")
                    nc.sync.dma_start(out=xt[:], in_=xr[128 * k:128 * (k + 1), nn, hh, :])
                    xq.append(xt)
                for s in range(4):      # 512-col sub-chunks
                    pp = ps2.tile([128, 512], FP32, tag="proj")
                    for k in range(4):
                        nc.tensor.matmul(
                            pp[:], lhsT=wts[k][:, 128 * h:128 * (h + 1)],
                            rhs=xq[k][:, 512 * s:512 * (s + 1)],
                            start=(k == 0), stop=(k == 3))
                    nc.scalar.activation(
                        qkv[:, 2048 * q + 512 * s: 2048 * q + 512 * (s + 1)], pp[:],
                        AF.Identity, bias=bia_s[:, h:h + 1], scale=scl_s[:, h:h + 1])

            # ---------- ext = [k' at 0:32 | q' at 32:64] ----------
            ext = big.tile([64, F], BF, tag="ext")
            nc.vector.tensor_copy(ext[0:32, :], qkv[32:64, :])
            nc.vector.tensor_copy(ext[32:64, :], qkv[0:32, :])
            extr = ext.rearrange("p (n h w) -> p n h w", n=2, h=64)

            # ---------- vT (PE pair-transposes) ----------
            vT = big.tile([128, 4096], BF, tag="vT")
            for b8 in range(8):         # 8 banks x 8 pairs
                pv = ps2.tile([128, 512], FP32, tag="proj")
                for tt in range(8):
                    t = b8 * 8 + tt
                    nn, wp = t // 32, t % 32
                    for par in range(2):
                        nc.tensor.matmul(
                            pv[64 * par:64 * (par + 1), 64 * tt:64 * (tt + 1)],
                            lhsT=qkvr[64:128, nn, :, 2 * wp + par],
                            rhs=idb_s[64:128, :],
                            start=True, stop=True)
                nc.vector.tensor_copy(vT[:, 512 * b8:512 * (b8 + 1)], pv[:])

            # ---------- qr (per-i batched) ----------
            qr_raw = big.tile([64, F], FP32, tag="qr")
            a21 = float(_emit.a2[h] / _emit.a1[h])
            for i4 in range(16):
                pq = ps2.tile([64, 512], FP32, tag="qrkr")
                for ii in range(4):
                    i = i4 * 4 + ii
                    nc.tensor.matmul(
                        pq[:, 128 * ii:128 * (ii + 1)],
                        lhsT=rel_s[0:32, 63 - i:127 - i],
                        rhs=qkvr[0:32, :, i, :], start=True, stop=True)
                nc.vector.tensor_scalar_mul(
                    qr_raw[:, 512 * i4:512 * (i4 + 1)], pq[:], a21)

            # ---------- kr (per-j batched) ----------
            kr_sb = big.tile([64, F], FP32, tag="kr")
            a3 = float(_emit.a3[h])
            for j4 in range(16):
                pk = ps2.tile([64, 512], FP32, tag="qrkr")
                for jj in range(4):
                    j = j4 * 4 + jj
                    nc.tensor.matmul(
                        pk[:, 128 * jj:128 * (jj + 1)],
                        lhsT=rel_s[32:64, 63 - j:127 - j],
                        rhs=qkvr[32:64, :, j, :], start=True, stop=True)
                nc.vector.tensor_scalar_mul(
                    kr_sb[:, 512 * j4:512 * (j4 + 1)], pk[:], a3)
            krr = kr_sb.rearrange("p (j n w) -> p j n w", j=64, n=2)
            qrw = qr_raw.rearrange("p (i n w) -> p n w i", i=64, n=2)

            # ---------- scores + softmax ----------
            E = big.tile([128, 4096], BF, tag="E")
            En = big.tile([128, 4096], BF, tag="En")
            for B in range(8):          # banks of 8 chunks; n = B // 4
                nn = B // 4
                wp0 = (B % 4) * 8
                pST = ps2.tile([128, 512], FP32, tag="ST")
                for tt in range(8):
                    wp = wp0 + tt
                    blk = pST[:, 64 * tt:64 * (tt + 1)]
                    # kr^T accumulate (f32 matmul against identity), per bg
                    nc.tensor.matmul(
                        blk[0:64, :], lhsT=krr[0:64, :, nn, 2 * wp],
                        rhs=idf_s[:], start=True, stop=False,
                        skip_group_check=True)
                    nc.tensor.matmul(
                        blk[64:128, :], lhsT=krr[0:64, :, nn, 2 * wp + 1],
                        rhs=idf_s[:], start=True, stop=False,
                        skip_group_check=True)
                    # qk even bg (w = 2wp): k' home, q' from ext (base 32)
                    nc.tensor.matmul(
                        blk[0:64, :], lhsT=qkvr[32:64, nn, :, 2 * wp],
                        rhs=extr[32:64, nn, :, 2 * wp],
                        start=False, stop=True, skip_group_check=True)
                    # qk odd bg (w = 2wp+1): k' copy (base 0), q' home
                    nc.tensor.matmul(
                        blk[64:128, :], lhsT=extr[0:32, nn, :, 2 * wp + 1],
                        rhs=qkvr[0:32, nn, :, 2 * wp + 1],
                        start=False, stop=True, skip_group_check=True)
                S_u = med.tile([128, 512], FP32, tag="Su")
                nc.vector.scalar_tensor_tensor(
                    S_u[0:64, :], pST[0:64, :], 1.0,
                    qrw[0:64, nn, 2 * wp0:2 * wp0 + 16:2, :],
                    op0=ALU.mult, op1=ALU.add)
                nc.vector.scalar_tensor_tensor(
                    S_u[64:128, :], pST[64:128, :], 1.0,
                    qrw[0:64, nn, 2 * wp0 + 1:2 * wp0 + 16:2, :],
                    op0=ALU.mult, op1=ALU.add)
                nc.scalar.activation(E[:, 512 * B:512 * (B + 1)], S_u[:], AF.Exp)
                # Z = column sums over j per half, 1/Z, broadcast via K=1 matmul
                pZ = ps2.tile([33, 512], FP32, tag="Z")
                nc.tensor.matmul(pZ[:], lhsT=on2_s[:], rhs=E[:, 512 * B:512 * (B + 1)],
                                 start=True, stop=True)
                Zlo = med.tile([1, 512], BF, tag="Zlo")
                Zhi = med.tile([1, 512], BF, tag="Zhi")
                with nc.allow_low_precision(reason="softmax 1/Z in bf16 is fine"):
                    nc.vector.reciprocal(Zlo[:], pZ[0:1, :])
                    nc.vector.reciprocal(Zhi[:], pZ[32:33, :])
                pZb = ps2.tile([128, 512], FP32, tag="Zb")
                nc.tensor.matmul(pZb[0:64, :], lhsT=onesb[:], rhs=Zlo[:],
                                 start=True, stop=True)
                nc.tensor.matmul(pZb[64:128, :], lhsT=onesb[:], rhs=Zhi[:],
                                 start=True, stop=True)
                nc.vector.tensor_mul(En[:, 512 * B:512 * (B + 1)],
                                     E[:, 512 * B:512 * (B + 1)], pZb[:])

            Eni = En.rearrange("p (t i) -> p i t", t=64)
            O = big.tile([64, F], BF, tag="O")
            Osv = O.rearrange("p (n h w) -> p n w h", n=2, h=64)
            Osve = O.rearrange("p (n h w) -> p h n w", n=2, h=64)

            # ---------- sv ----------
            for B in range(8):          # bank: 16 bg = one n, w in [16B'..]
                nn = B // 4
                wp0 = (B % 4) * 8
                pSV = ps2.tile([128, 512], FP32, tag="SV")
                for tt in range(8):
                    for par in range(2):
                        t = nn * 32 + wp0 + tt
                        nc.tensor.matmul(
                            pSV[64 * par:64 * (par + 1), 64 * tt:64 * (tt + 1)],
                            lhsT=vT[64 * par:64 * (par + 1), 64 * t:64 * (t + 1)],
                            rhs=En[64 * par:64 * (par + 1), 64 * t:64 * (t + 1)],
                            start=True, stop=True)
                for par in range(2):
                    nc.vector.tensor_scalar(
                        Osv[0:64, nn, 2 * wp0 + par:2 * wp0 + 16:2, :],
                        pSV[64 * par:64 * (par + 1), :].rearrange(
                            "p (t i) -> p t i", t=8),
                        aE_s[64 * par:64 * par + 64, h:h + 1],
                        bb_s[64 * par:64 * par + 64, h:h + 1],
                        op0=ALU.mult, op1=ALU.add)

            # ---------- sve ----------
            for ig in range(8):         # 8 i's x 2 par per bank
                pSE = ps2.tile([128, 512], FP32, tag="SV")
                for ii in range(8):
                    i = ig * 8 + ii
                    for par in range(2):
                        nc.tensor.matmul(
                            pSE[64 * par:64 * (par + 1), 64 * ii:64 * (ii + 1)],
                            lhsT=vet_s[64 * par:64 * (par + 1), 64 * i:64 * (i + 1)],
                            rhs=Eni[64 * par:64 * (par + 1), i, :],
                            start=True, stop=True)
                for par in range(2):
                    for nn2 in range(2):
                        nc.vector.scalar_tensor_tensor(
                            Osve[0:64, 8 * ig:8 * (ig + 1), nn2, par::2],
                            pSE[64 * par:64 * (par + 1), :].rearrange(
                                "p (ii n wp) -> p ii n wp", ii=8, n=2)[:, :, nn2, :],
                            aO_s[0:64, h:h + 1],
                            Osve[0:64, 8 * ig:8 * (ig + 1), nn2, par::2],
                            op0=ALU.mult, op1=ALU.add)

            # ---------- DMA out ----------
            Ond = O.rearrange("p (n hw) -> p n hw", n=2)
            nc.sync.dma_start(out=oer[64 * h:64 * (h + 1), :, :], in_=Ond[:, :, :])


def _fold_constants(w_qkv, relative,
                    bnq_g, bnq_b, bnq_m, bnq_v,
                    bns_g, bns_b, bns_m, bns_v,
                    bno_g, bno_b, bno_m, bno_v):
    s_q = bnq_g / np.sqrt(bnq_v + BN_EPS)
    t_q = bnq_b - bnq_m * s_q
    a_s = bns_g / np.sqrt(bns_v + BN_EPS)
    a1, a2, a3 = a_s[0:8], a_s[8:16], a_s[16:24]
    s_o = bno_g / np.sqrt(bno_v + BN_EPS)
    t_o = bno_b - bno_m * s_o

    scl = np.empty((128, 8), np.float32)
    bia = np.empty((128, 8), np.float32)
    for h in range(8):
        oc = h * 128 + np.arange(128)
        f = np.where(np.arange(128) < 32, a1[h], 1.0)
        scl[:, h] = s_q[oc] * f
        bia[:, h] = t_q[oc] * f

    rel_t = np.empty((64, 127), np.float32)
    rel_t[0:32] = relative[0:32, ::-1]       # relq reversed
    rel_t[32:64] = relative[32:64, ::-1]     # relk reversed

    # v_embT[par*64+j, i*64+c] = relative[64+c, i-j+63]
    iidx = np.arange(64)
    v_embT = np.empty((128, 4096), np.float32)
    jj, ii_, cc = np.meshgrid(np.arange(64), iidx, np.arange(64), indexing="ij")
    tbl = relative[64 + cc, ii_ - jj + 63]   # (j, i, c)
    v_embT[0:64] = tbl.reshape(64, 4096)
    v_embT[64:128] = tbl.reshape(64, 4096)

    ones2 = np.zeros((128, 33), np.float32)
    ones2[0:64, 0] = 1.0
    ones2[64:128, 32] = 1.0
    idb = np.zeros((128, 64), np.float32)
    idb[np.arange(128), np.arange(128) % 64] = 1.0
    idf = np.eye(64, dtype=np.float32)

    aE = np.empty((128, 8), np.float32)
    aO = np.empty((128, 8), np.float32)
    bb = np.empty((128, 8), np.float32)
    for h in range(8):
        cc_ = np.arange(128) % 64
        oc2 = (h * 64 + cc_) * 2
        aE[:, h] = s_o[oc2]
        aO[:, h] = s_o[oc2 + 1]
        bb[:, h] = t_o[oc2] + t_o[oc2 + 1]

    wt = np.ascontiguousarray(w_qkv.T)
    return dict(wt=wt.astype(BF16), scl=scl, bia=bia,
                rel_t=np.ascontiguousarray(rel_t).astype(BF16),
                v_embT=v_embT.astype(BF16), ones2=ones2.astype(BF16),
                idb=idb.astype(BF16), idf=idf,
                aE=aE, aO=aO, bb=bb, a1=a1, a2=a2, a3=a3)


def _build_graph(a1, a2, a3):
    nc = bacc.Bacc("TRN2")
    xi = nc.dram_tensor("x", (2, 512, 64, 64), BF, kind="ExternalInput")
    wt = nc.dram_tensor("wt", (512, 1024), BF, kind="ExternalInput")
    scl = nc.dram_tensor("scl", (128, 8), FP32, kind="ExternalInput")
    bia = nc.dram_tensor("bia", (128, 8), FP32, kind="ExternalInput")
    rel_t = nc.dram_tensor("rel_t", (64, 127), BF, kind="ExternalInput")
    v_embT = nc.dram_tensor("v_embT", (128, 4096), BF, kind="ExternalInput")
    ones2 = nc.dram_tensor("ones2", (128, 33), BF, kind="ExternalInput")
    idb = nc.dram_tensor("idb", (128, 64), BF, kind="ExternalInput")
    idf = nc.dram_tensor("idf", (64, 64), FP32, kind="ExternalInput")
    aE = nc.dram_tensor("aE", (128, 8), FP32, kind="ExternalInput")
    aO = nc.dram_tensor("aO", (128, 8), FP32, kind="ExternalInput")
    bb = nc.dram_tensor("bb", (128, 8), FP32, kind="ExternalInput")
    sc2 = nc.dram_tensor("sc2", (1, 8), FP32, kind="ExternalInput")
    oe = nc.dram_tensor("o", (2, 512, 64, 64), BF, kind="ExternalOutput")

    _emit.a1, _emit.a2, _emit.a3 = a1, a2, a3
    with tile.TileContext(nc) as tc:
        _emit(tc, xi[:], wt[:], scl[:], bia[:], rel_t[:], v_embT[:], ones2[:],
              idb[:], idf[:], aE[:], aO[:], bb[:], sc2[:], oe[:])
    nc.compile()
    return nc


_LAST_EXEC_NS = None


def _mesh_sharding():
    import jax
    from jax.sharding import Mesh, PartitionSpec, NamedSharding
    devices = jax.devices()[:NCORES]
    mesh = Mesh(np.asarray(devices), ("core",))
    return mesh, NamedSharding(mesh, PartitionSpec("core"))


def _run_spmd_fast(nc, global_ins):
    """Execute nc on 8 cores via shard_map with on-device zero outputs and
    pre-transferred (async) inputs. global_ins: name -> global array whose
    axis 0 is ncores * per_core_dim0."""
    import jax
    import jax.numpy as jnp
    from jax.sharding import Mesh, PartitionSpec, NamedSharding
    from concourse import bass2jax

    bass2jax.install_neuronx_cc_hook()
    partition_name = (nc.partition_id_tensor.name
                      if nc.partition_id_tensor else None)
    in_names, out_names, out_avals = [], [], []
    for alloc in nc.m.functions[0].allocations:
        if not isinstance(alloc, mybir.MemoryLocationSet):
            continue
        name = alloc.memorylocations[0].name
        if alloc.kind == "ExternalInput":
            if name != partition_name:
                in_names.append(name)
        elif alloc.kind == "ExternalOutput":
            shape = tuple(alloc.tensor_shape)
            dtype = mybir.dt.np(alloc.dtype)
            out_names.append(name)
            out_avals.append(jax.core.ShapedArray(shape, dtype))
    n_params = len(in_names)
    n_outs = len(out_avals)
    # No zero output buffers: this kernel writes every output element, so
    # uninitialized PJRT result buffers are fine (saves their wire transfer).
    all_in_names = list(in_names)
    if partition_name is not None:
        all_in_names.append(partition_name)

    mesh, sh = _mesh_sharding()

    # inputs were device_put by the caller (async, overlapped with build)
    dev_in = [global_ins[nm] for nm in in_names]

    def _body(*args):
        operands = list(args)
        if partition_name is not None:
            operands.append(bass2jax.partition_id_tensor())
        outs = bass2jax._bass_exec_p.bind(
            *operands,
            out_avals=tuple(out_avals),
            in_names=tuple(all_in_names),
            out_names=tuple(out_names),
            lowering_input_output_aliases=(),
            sim_require_finite=True,
            sim_require_nnan=True,
            nc=nc,
        )
        return tuple(outs)

    in_specs = (PartitionSpec("core"),) * n_params
    out_specs = (PartitionSpec("core"),) * n_outs
    sharded = jax.jit(
        bass2jax.shard_map(_body, mesh=mesh, in_specs=in_specs,
                           out_specs=out_specs, check_rep=False),
        keep_unused=True)
    out_arrs = sharded(*dev_in)
    return {nm: np.asarray(out_arrs[i]) for i, nm in enumerate(out_names)}


def kernel(x, w_qkv, relative,
           bnq_g, bnq_b, bnq_m, bnq_v,
           bns_g, bns_b, bns_m, bns_v,
           bno_g, bno_b, bno_m, bno_v):
    global _LAST_EXEC_NS
    cst = _fold_constants(np.asarray(w_qkv, np.float32), np.asarray(relative, np.float32),
                          *[np.asarray(a, np.float32) for a in
                            (bnq_g, bnq_b, bnq_m, bnq_v, bns_g, bns_b, bns_m, bns_v,
                             bno_g, bno_b, bno_m, bno_v)])
    x_bf = np.ascontiguousarray(np.asarray(x, np.float32)).astype(BF16)
    global_ins = {"x": x_bf, "sc2": np.zeros((NCORES, 8), np.float32)}
    for k in ("wt", "scl", "bia", "rel_t", "v_embT", "ones2",
              "idb", "idf", "aE", "aO", "bb"):
        a = cst[k]
        global_ins[k] = np.tile(a, (NCORES,) + (1,) * (a.ndim - 1))
    # kick off async H2D now so the transfer overlaps graph build + compile.
    # Use per-device puts + make_array (the NamedSharding device_put path is
    # drastically slower and occasionally pathological through axon).
    import jax
    _, sh = _mesh_sharding()
    devs = jax.devices()[:NCORES]
    # tiny per-device warmup: first transfer on a cold channel runs ~4x
    # slower; a small put first moves the big ones onto the warm path
    _tiny = np.ones((8, 128), BF16)
    for _w in [jax.device_put(_tiny, dv) for dv in devs]:
        _w.block_until_ready()

    def _put_global(arr):
        per = arr.shape[0] // NCORES
        shards = [jax.device_put(arr[r * per:(r + 1) * per], devs[r])
                  for r in range(NCORES)]
        return jax.make_array_from_single_device_arrays(arr.shape, sh, shards)

    global_ins = {k: _put_global(v) for k, v in global_ins.items()}
    # drain transfers BEFORE the python-heavy graph build: in-flight axon
    # transfers stall (sometimes catastrophically, with retry blowups) while
    # the main thread is busy, so do not overlap them with build/compile.
    for _v in global_ins.values():
        _v.block_until_ready()
    nc = _build_graph(cst["a1"], cst["a2"], cst["a3"])
    res = _run_spmd_fast(nc, global_ins)
    _LAST_EXEC_NS = None
    return res["o"].astype(np.float32)
